# revision 1
# baseline (speedup 1.0000x reference)
"""Causal self-attention (B=2, T=4096, C=768, H=12, D=64) on 8 trn2 cores.

Sharding: batch*heads across cores. Core c handles batch c//4 and heads
3*(c%4) .. 3*(c%4)+2. Each core computes the QKV projection for its head
slice, full causal attention for those heads, and a partial output
projection (its heads' rows of w_out). The host sums the 4 partials per
batch and adds b_out.

On-core layouts (matmul operands float32r - fp32 data consumed at full
PE rate with ~1e-4 rounding; the PE rounds on read, so fp32 bits are
DMA'd straight into f32r tiles):
  xT      [C, T]   input, pre-transposed on host
  qT/kT   [64, T]  packed so q_h and k_h share a partition base
                   (matmul requires lhsT/rhs base alignment)
  v_aug   [T, 256] natural-layout v with a ones column per head at
                   col h*65+64 (so P@V also yields softmax denominators)
  scoresT [k, q]   psum; exp on ACT; causal mask via gpsimd affine_select
  outT    [65, q]  psum accumulation over k tiles; row 64 = sum(exp)

Packed [128, T] sbuf tiles (rows 0:64 | 64:128):
  tA = [qT_h0 | qT_h1]   tB = [kT_h0 | kT_h1]
  tC = [outT_h0 | outT_h1] tD = [outT_h2 | qT_h2] tE = [- | kT_h2]
(outT_h0/h1 share a tile so the output projection contracts 128 rows
per matmul; h1's normalize result is DMA-bounced to partition base 64)

The projection is emitted chunk-by-chunk inside the attention q-block
loop (chunk qb produces exactly the 512 columns attention block qb
needs), so the scalar engine's exp stream starts as soon as the first
chunk lands instead of after the whole projection.
"""

import numpy as np

import concourse.bass as bass
import concourse.mybir as mybir
import concourse.tile as tile
from concourse import bacc
from concourse.bass_utils import run_bass_kernel_spmd

B, T, C = 2, 4096, 768
NH, D = 12, 64
HPC = 3  # heads per core
NCORES = 8
P = 128
QB = 512           # q block == projection chunk
NQB = T // QB      # 8
NKT = T // P       # 32 k tiles
F32 = mybir.dt.float32
F32R = mybir.dt.float32r

_CACHE = {}


def _build_nc():
    nc = bacc.Bacc(
        "TRN2",
        target_bir_lowering=False,
        debug=False,
        enable_asserts=False,
        num_devices=NCORES,
    )
    # wqk columns: [q_h0 q_h1 | k_h0 k_h1 | q_h2 | k_h2]
    xT = nc.dram_tensor("xT", [C, T], F32R, kind="ExternalInput")
    wqk = nc.dram_tensor("wqk", [C, 2 * HPC * D], F32R, kind="ExternalInput")
    wv = nc.dram_tensor("wv", [C, 256], F32R, kind="ExternalInput")
    wo = nc.dram_tensor("wo", [HPC * D, C], F32R, kind="ExternalInput")
    out = nc.dram_tensor("out", [T, C], F32, kind="ExternalOutput")

    with tile.TileContext(nc) as tc:
        _emit(tc, nc, xT.ap(), wqk.ap(), wv.ap(), wo.ap(), out.ap())
    nc.compile()
    return nc


def _emit(tc, nc, xT, wqk, wv, wo, out):
    import contextlib

    ctx = contextlib.ExitStack()
    with ctx:
        # ---- persistent sbuf ----
        persist = ctx.enter_context(tc.tile_pool(name="persist", bufs=1))
        packs = [
            persist.tile([P, T], F32R, tag=f"pk{m}", name=f"pk{m}") for m in range(5)
        ]
        tA, tB, tC, tD, tE = packs
        vaug = persist.tile([P, NKT, 256], F32R, tag="vaug")
        wqk_sb = persist.tile([P, 6, 2 * HPC * D], F32R, tag="wqk")
        wv_sb = persist.tile([P, 6, 256], F32R, tag="wv")
        wo01_sb = persist.tile([P, C], F32R, tag="wo01")
        wo2_sb = persist.tile([D, C], F32R, tag="wo2")
        ones_f32 = persist.tile([P, D], F32, tag="onesf32")

        nc.sync.dma_start(out=wqk_sb[:], in_=wqk.rearrange("(co p) n -> p co n", p=P))
        nc.sync.dma_start(out=wv_sb[:], in_=wv.rearrange("(co p) n -> p co n", p=P))
        nc.sync.dma_start(out=wo01_sb[:], in_=wo[0:P, :])
        nc.sync.dma_start(out=wo2_sb[:], in_=wo[P : P + D, :])
        nc.gpsimd.memset(ones_f32[:], 1.0)

        def qT(h):
            return (tA[0:D], tA[D:P], tD[D:P])[h]

        def kT(h):
            return (tB[0:D], tB[D:P], tE[D:P])[h]

        # ---- fused projection + attention loop ----
        # psum budget (8 banks): p1 2 + scores 4 + outT 2
        with (
            tc.tile_pool(name="xchunks", bufs=2) as xpool,
            tc.tile_pool(name="p1psum", bufs=2, space="PSUM") as p1psum,
            tc.tile_pool(name="spsum", bufs=3, space="PSUM") as spool,
            tc.tile_pool(name="p3psum", bufs=1, space="PSUM") as p3psum,
            tc.tile_pool(name="opsum", bufs=2, space="PSUM") as opool,
            tc.tile_pool(name="exps", bufs=3) as epool,
            tc.tile_pool(name="smalls", bufs=4) as rpool,
            tc.tile_pool(name="dscratch", bufs=4, space="DRAM") as dpool,
        ):
            for qb in range(NQB):
                qsl = slice(qb * QB, (qb + 1) * QB)

                # -- projection chunk qb: columns [qb*512, qb*512+512) --
                xt = xpool.tile([P, 6, QB], F32R, tag="xt")
                nc.sync.dma_start(
                    out=xt[:], in_=xT[:, qsl].rearrange("(co p) t -> p co t", p=P)
                )
                for ci in range(3):
                    ps = p1psum.tile([P, QB], F32, tag="p1", name=f"p1_{qb}_{ci}")
                    for c6 in range(6):
                        nc.tensor.matmul(
                            ps[:],
                            wqk_sb[:, c6, ci * P : (ci + 1) * P],
                            xt[:, c6, :],
                            start=(c6 == 0),
                            stop=(c6 == 5),
                        )
                    if ci < 2:
                        dst = (tA, tB)[ci]
                        nc.vector.tensor_copy(out=dst[:, qsl], in_=ps[:])
                    else:
                        # chain 2 = [qT_h2 | kT_h2] at psum base 0; the packed
                        # destinations live at partition base 64, which only a
                        # DMA can reach (engines cannot cross partitions)
                        stg = xpool.tile([P, QB], F32R, tag="stg")
                        nc.vector.tensor_copy(out=stg[:], in_=ps[:])
                        nc.sync.dma_start(out=tD[D:P, qsl], in_=stg[0:D, :])
                        nc.sync.dma_start(out=tE[D:P, qsl], in_=stg[D:P, :])
                for half in range(QB // P):
                    ktv = qb * (QB // P) + half
                    ps2 = p1psum.tile([P, QB], F32, tag="p1", name=f"p1v_{qb}_{half}")
                    for c6 in range(6):
                        nc.tensor.matmul(
                            ps2[:, 0:256],
                            xt[:, c6, half * P : (half + 1) * P],
                            wv_sb[:, c6, :],
                            start=(c6 == 0),
                            stop=(c6 == 5),
                        )
                    nc.vector.tensor_copy(out=vaug[:, ktv, :], in_=ps2[:, 0:256])
                # restore the ones columns the v copies just overwrote
                for h in range(HPC):
                    nc.vector.tensor_copy(
                        out=vaug[:, qb * (QB // P) : (qb + 1) * (QB // P),
                                 h * (D + 1) + D],
                        in_=ones_f32[:, 0 : QB // P],
                    )

                # -- attention for q block qb --
                for h in range(HPC):
                    nkt = 4 * qb + 4
                    outp = opool.tile([D + 1, QB], F32, tag="outT")
                    for kt in range(nkt):
                        co = max(0, P * (kt - 4 * qb))
                        sp = spool.tile([P, QB], F32, tag="scores")
                        nc.tensor.matmul(
                            sp[:, co:],
                            kT(h)[:, kt * P : (kt + 1) * P],
                            qT(h)[:, qb * QB + co : (qb + 1) * QB],
                            start=True,
                            stop=True,
                        )
                        ex = epool.tile([P, QB], F32R, tag="ex")
                        nc.scalar.activation(
                            out=ex[:, co:],
                            in_=sp[:, co:],
                            func=mybir.ActivationFunctionType.Exp,
                            scale=float(D) ** -0.5,
                        )
                        if kt >= 4 * qb:  # diagonal band: causal mask
                            nc.gpsimd.affine_select(
                                out=ex[:, co:],
                                in_=ex[:, co:],
                                compare_op=mybir.AluOpType.is_ge,
                                fill=0.0,
                                base=0,
                                pattern=[[1, QB - co]],
                                channel_multiplier=-1,
                            )
                        nc.tensor.matmul(
                            outp[:, co:],
                            vaug[:, kt, h * (D + 1) : (h + 1) * (D + 1)],
                            ex[:, co:],
                            start=(kt == 0),
                            stop=(kt == nkt - 1),
                        )
                    # softmax denominators: reciprocal of outp row 64 stays at
                    # partition base 64 (engines cannot cross partitions); a
                    # partition-broadcast DMA then fans it out across 0:64
                    recip = rpool.tile([D + 1, QB], F32, tag="recip")
                    nc.vector.reciprocal(
                        out=recip[D : D + 1, :], in_=outp[D : D + 1, :]
                    )
                    # partition-broadcast via DRAM bounce (SBUF sources must
                    # have nonzero partition step; DRAM reads may broadcast)
                    dsc = dpool.tile([1, QB], F32, tag="dsc")
                    nc.sync.dma_start(out=dsc[:], in_=recip[D : D + 1, :])
                    bcs = rpool.tile([D, QB], F32, tag="bcs")
                    nc.gpsimd.dma_start(
                        out=bcs[:],
                        in_=bass.AP(
                            tensor=dsc.tensor,
                            offset=dsc.offset,
                            ap=[[0, D]] + list(dsc.ap[-1:]),
                        ),
                    )
                    if h == 0:
                        nc.vector.tensor_mul(
                            out=tC[0:D, qsl], in0=outp[0:D, :], in1=bcs[:]
                        )
                    elif h == 2:
                        nc.vector.tensor_mul(
                            out=tD[0:D, qsl], in0=outp[0:D, :], in1=bcs[:]
                        )
                    else:
                        # h1 lives at partition base 64 of tC; engines cannot
                        # cross partitions, so normalize into a staging tile
                        # and DMA-bounce it up
                        ot = rpool.tile([D, QB], F32R, tag="otmp", bufs=2)
                        nc.vector.tensor_mul(
                            out=ot[:], in0=outp[0:D, :], in1=bcs[:]
                        )
                        nc.sync.dma_start(out=tC[D:P, qsl], in_=ot[:])

                # -- output projection for this q block (tail of the loop;
                # psum comes from the p1 tag so the bank budget stays at 8) --
                for tt in range(qb * (QB // P), (qb + 1) * (QB // P)):
                    tsl = slice(tt * P, (tt + 1) * P)
                    so = rpool.tile([P, C], F32, tag="p3out", bufs=2)
                    for noff, nsz in ((0, 512), (512, 256)):
                        po = p3psum.tile(
                            [P, QB], F32, tag="p3", name=f"po_{tt}_{noff}"
                        )
                        nc.tensor.matmul(
                            po[:, :nsz],
                            tC[:, tsl],
                            wo01_sb[:, noff : noff + nsz],
                            start=True,
                            stop=False,
                        )
                        nc.tensor.matmul(
                            po[:, :nsz],
                            tD[0:D, tsl],
                            wo2_sb[:, noff : noff + nsz],
                            start=False,
                            stop=True,
                        )
                        nc.vector.tensor_copy(
                            out=so[:, noff : noff + nsz], in_=po[:, :nsz]
                        )
                    nc.sync.dma_start(out=out[tsl, :], in_=so[:])


def _get_nc():
    if "nc" not in _CACHE:
        _CACHE["nc"] = _build_nc()
    return _CACHE["nc"]


def _shard_inputs(x, w_qkv, w_out):
    """Build per-core input maps."""
    x = np.asarray(x, dtype=np.float32)
    w_qkv = np.asarray(w_qkv, dtype=np.float32)
    w_out = np.asarray(w_out, dtype=np.float32)
    xTs = [np.ascontiguousarray(x[b].T) for b in range(B)]
    in_maps = []
    for c in range(NCORES):
        b = c // 4
        heads = [HPC * (c % 4) + i for i in range(HPC)]
        q = [w_qkv[:, h * D : (h + 1) * D] for h in heads]
        k = [w_qkv[:, C + h * D : C + (h + 1) * D] for h in heads]
        wqk = np.concatenate([q[0], q[1], k[0], k[1], q[2], k[2]], axis=1)
        wv = np.zeros((C, 256), dtype=np.float32)
        for i, h in enumerate(heads):
            wv[:, i * (D + 1) : i * (D + 1) + D] = w_qkv[
                :, 2 * C + h * D : 2 * C + (h + 1) * D
            ]
        wo = np.concatenate(
            [w_out[h * D : (h + 1) * D, :] for h in heads], axis=0
        )
        in_maps.append(
            {
                "xT": xTs[b],
                "wqk": np.ascontiguousarray(wqk),
                "wv": wv,
                "wo": np.ascontiguousarray(wo),
            }
        )
    return in_maps


def kernel(x, w_qkv, w_out, b_out):
    nc = _get_nc()
    in_maps = _shard_inputs(x, w_qkv, w_out)
    res = run_bass_kernel_spmd(nc, in_maps, core_ids=list(range(NCORES)))
    b_out = np.asarray(b_out, dtype=np.float32)
    outs = []
    for b in range(B):
        acc = res.results[4 * b]["out"].astype(np.float32).copy()
        for c in range(4 * b + 1, 4 * b + 4):
            acc += res.results[c]["out"]
        outs.append(acc + b_out[None, :])
    return np.stack(outs, axis=0)



# revision 11
# speedup vs baseline: 1.2649x; 1.2649x over previous
"""Causal self-attention (B=2, T=4096, C=768, H=12, D=64) on 8 trn2 cores.

Sharding: batch*heads across cores. Core c handles batch c//4 and heads
3*(c%4) .. 3*(c%4)+2. Each core computes the QKV projection for its head
slice, full causal attention for those heads, and a partial output
projection (its heads' rows of w_out). The host sums the 4 partials per
batch and adds b_out.

On-core layouts (q/k matmul operands float32r - fp32 consumed at full PE
rate with ~1e-4 rounding; v / softmax weights / out-proj in bf16):
  xT      [C, T]    input, pre-transposed on host
  tA      [128, T]  = [qT_h0 | qT_h1]   (rows 0:64 | 64:128)
  tB      [128, T]  = [kT_h0 | kT_h1]
  tQ2/tK2 [128, T]  = [qT_h2 | qT_h2] / [kT_h2 | kT_h2]  (duplicated
                      halves so h2's scores can use either partition
                      base without a cross-partition copy)
  vaug    [128, NKT, 256] bf16, v with a ones column per head at
                      col h*65+64 (so P@V also yields softmax denoms)
  scores  [128, 1024] psum, TWO k-tiles per bank-pair so one ACT exp
                      instruction covers 1024 columns (amortizes the
                      ~350-cycle ACT instruction overhead)
  attnO   [64, 3, T] bf16 normalized attention outputs per head

h0 lives at partition base 0 and h1 at base 64, and their score matmuls
are emitted adjacently, so the PE can run them concurrently in disjoint
row-groups (K=64 each).

Softmax denominators: PV psum row 64 = sum(exp) -> DVE reciprocal ->
gpsimd partition_broadcast -> DVE multiply. No DRAM round trips.

The causal mask is applied to the exp output of the 4 diagonal k-tiles
per q-block with gpsimd affine_select (fills 0 where q < k, including
the never-computed stale region of the staircase).
"""

import numpy as np
from ml_dtypes import bfloat16

import concourse.bass as bass
import concourse.mybir as mybir
import concourse.tile as tile
from concourse import bacc
from concourse.bass_utils import run_bass_kernel_spmd

B, T, C = 2, 4096, 768
NH, D = 12, 64
HPC = 3  # heads per core
NCORES = 8
P = 128
QB = 512           # q block == projection chunk
NQB = T // QB      # 8
NKT = T // P       # 32 k tiles
F32 = mybir.dt.float32
F32R = mybir.dt.float32r
BF16 = mybir.dt.bfloat16

_CACHE = {}


def _declare(nc):
    xT = nc.dram_tensor("xT", [C, T], F32R, kind="ExternalInput")
    wqk = nc.dram_tensor("wqk", [C, 4 * P], F32R, kind="ExternalInput")
    wv = nc.dram_tensor("wv", [C, 256], F32R, kind="ExternalInput")
    wo = nc.dram_tensor("wo", [D, HPC * C], BF16, kind="ExternalInput")
    out = nc.dram_tensor("out", [T, C], F32, kind="ExternalOutput")
    return dict(xT=xT.ap(), wqk=wqk.ap(), wv=wv.ap(), wo=wo.ap(), out=out.ap())


def _build_nc():
    nc = bacc.Bacc(
        "TRN2",
        target_bir_lowering=False,
        debug=False,
        enable_asserts=False,
        num_devices=NCORES,
    )
    aps = _declare(nc)
    with tile.TileContext(nc) as tc:
        _emit(tc, nc, **aps)
    nc.compile()
    return nc


def _emit(tc, nc, xT, wqk, wv, wo, out):
    import contextlib

    ctx = contextlib.ExitStack()
    with ctx:
        # ---- persistent sbuf ----
        persist = ctx.enter_context(tc.tile_pool(name="persist", bufs=1))
        tA = persist.tile([P, T], F32R, tag="tA")
        tB = persist.tile([P, T], F32R, tag="tB")
        tQ2 = persist.tile([P, T], F32R, tag="tQ2")
        tK2 = persist.tile([P, T], F32R, tag="tK2")
        vaug = persist.tile([P, NKT, 256], BF16, tag="vaug")
        attnO = persist.tile([D, HPC, T], BF16, tag="attnO")
        wqk_sb = persist.tile([P, 6, 4 * P], F32R, tag="wqk")
        wv_sb = persist.tile([P, 6, 256], F32R, tag="wv")
        wo_sb = persist.tile([D, HPC, C], BF16, tag="wo")
        ones_bf = persist.tile([P, 8], BF16, tag="ones")

        nc.sync.dma_start(out=wqk_sb[:], in_=wqk.rearrange("(co p) n -> p co n", p=P))
        nc.sync.dma_start(out=wv_sb[:], in_=wv.rearrange("(co p) n -> p co n", p=P))
        nc.sync.dma_start(out=wo_sb[:], in_=wo)
        nc.gpsimd.memset(ones_bf[:], 1.0)

        def qT(h, base=0):
            # h0 rows 0:64 of tA, h1 rows 64:128 of tA; h2 lives duplicated
            # in both halves of tQ2 so either partition base works
            return (tA[0:D], tA[D:P], tQ2[base : base + D])[h]

        def kT(h, base=0):
            return (tB[0:D], tB[D:P], tK2[base : base + D])[h]

        qk_dst = (tA, tB, tQ2, tK2)

        # psum budget (8 banks): p1 2 + scores 2*2 + outT 2
        with (
            tc.tile_pool(name="xchunks", bufs=2) as xpool,
            tc.tile_pool(name="p1psum", bufs=2, space="PSUM") as p1psum,
            tc.tile_pool(name="spsum", bufs=2, space="PSUM") as spool,
            tc.tile_pool(name="opsum", bufs=2, space="PSUM") as opool,
            tc.tile_pool(name="exps", bufs=3) as epool,
            tc.tile_pool(name="smalls", bufs=4) as rpool,
        ):
            from collections import deque

            def proj_work(qb):
                """Closures emitting projection chunk qb (tokens qb*512..)."""
                qsl = slice(qb * QB, (qb + 1) * QB)
                st = {}

                def dma():
                    xt = xpool.tile([P, 6, QB], F32R, tag="xt")
                    nc.sync.dma_start(
                        out=xt[:],
                        in_=xT[:, qsl].rearrange("(co p) t -> p co t", p=P),
                    )
                    st["xt"] = xt

                def chain(ci):
                    def f():
                        ps = p1psum.tile(
                            [P, QB], F32, tag="p1", name=f"p1_{qb}_{ci}"
                        )
                        for c6 in range(6):
                            nc.tensor.matmul(
                                ps[:],
                                wqk_sb[:, c6, ci * P : (ci + 1) * P],
                                st["xt"][:, c6, :],
                                start=(c6 == 0),
                                stop=(c6 == 5),
                            )
                        nc.vector.tensor_copy(out=qk_dst[ci][:, qsl], in_=ps[:])
                    return f

                def vhalf(half):
                    def f():
                        ktv = qb * (QB // P) + half
                        ps2 = p1psum.tile(
                            [P, QB], F32, tag="p1", name=f"p1v_{qb}_{half}"
                        )
                        for c6 in range(6):
                            nc.tensor.matmul(
                                ps2[:, 0:256],
                                st["xt"][:, c6, half * P : (half + 1) * P],
                                wv_sb[:, c6, :],
                                start=(c6 == 0),
                                stop=(c6 == 5),
                            )
                        nc.vector.tensor_copy(
                            out=vaug[:, ktv, :], in_=ps2[:, 0:256]
                        )
                        if half == QB // P - 1:
                            # restore the ones columns the v copies overwrote
                            for h in range(HPC):
                                nc.vector.tensor_copy(
                                    out=vaug[:, qb * (QB // P) :
                                             (qb + 1) * (QB // P),
                                             h * (D + 1) + D],
                                    in_=ones_bf[:, 0 : QB // P],
                                )
                    return f

                return (
                    [dma]
                    + [chain(ci) for ci in range(4)]
                    + [vhalf(h) for h in range(QB // P)]
                )

            def outproj_work(qb):
                """Closures emitting the output projection of q block qb."""
                def tt_work(tt):
                    def f():
                        tsl = slice(tt * P, (tt + 1) * P)
                        so = rpool.tile([P, C], F32, tag="p3out", bufs=2)
                        for noff, nsz in ((0, 512), (512, 256)):
                            po = p1psum.tile(
                                [P, QB], F32, tag="p1", name=f"po_{tt}_{noff}"
                            )
                            for h in range(HPC):
                                nc.tensor.matmul(
                                    po[:, :nsz],
                                    attnO[:, h, tsl],
                                    wo_sb[:, h, noff : noff + nsz],
                                    start=(h == 0),
                                    stop=(h == HPC - 1),
                                )
                            nc.vector.tensor_copy(
                                out=so[:, noff : noff + nsz], in_=po[:, :nsz]
                            )
                        nc.sync.dma_start(out=out[tsl, :], in_=so[:])
                    return f

                return [
                    tt_work(tt)
                    for tt in range(qb * (QB // P), (qb + 1) * (QB // P))
                ]

            def attn_group(qb, h, g, outp, base):
                """Two k-tiles (2g, 2g+1): scores -> one exp -> mask -> PV."""
                nkt = 4 * qb + 4
                sp = spool.tile([P, 2 * QB], F32, tag="sc",
                                name=f"sp_{qb}_{h}_{g}")
                for i in range(2):
                    kt = 2 * g + i
                    nc.tensor.matmul(
                        sp[:, i * QB : (i + 1) * QB],
                        kT(h, base)[:, kt * P : (kt + 1) * P],
                        qT(h, base)[:, qb * QB : (qb + 1) * QB],
                        start=True,
                        stop=True,
                    )
                ex = epool.tile([P, 2 * QB], BF16, tag="ex")
                nc.scalar.activation(
                    out=ex[:],
                    in_=sp[:],
                    func=mybir.ActivationFunctionType.Exp,
                    scale=float(D) ** -0.5,
                )
                for i in range(2):
                    kt = 2 * g + i
                    j = kt - 4 * qb
                    if j >= 0:  # diagonal tile: causal mask (fill 0 at q<k)
                        nc.gpsimd.affine_select(
                            out=ex[:, i * QB : (i + 1) * QB],
                            in_=ex[:, i * QB : (i + 1) * QB],
                            compare_op=mybir.AluOpType.is_ge,
                            fill=0.0,
                            base=-P * j,
                            pattern=[[1, QB]],
                            channel_multiplier=-1,
                        )
                for i in range(2):
                    kt = 2 * g + i
                    co = max(0, P * (kt - 4 * qb))
                    nc.tensor.matmul(
                        outp[:, co:],
                        vaug[:, kt, h * (D + 1) : (h + 1) * (D + 1)],
                        ex[:, i * QB + co : (i + 1) * QB],
                        start=(kt == 0),
                        stop=(kt == nkt - 1),
                    )

            def normalize(qb, hs, outps):
                """Softmax denominators for one or two heads at once: DVE
                reciprocal of psum row 64, one sbuf->sbuf DMA to move the
                rows to partition 0 (partition_broadcast's ucode reads
                physical partition 0 only), one gpsimd broadcast, then the
                normalizing multiplies."""
                qsl = slice(qb * QB, (qb + 1) * QB)
                nh = len(hs)
                rt = rpool.tile([D + 1, nh, QB], F32, tag=f"recip{nh}", bufs=2)
                for i, outp in enumerate(outps):
                    nc.vector.reciprocal(
                        out=rt[D : D + 1, i, :], in_=outp[D : D + 1, :]
                    )
                rb = rpool.tile([1, nh, QB], F32, tag=f"rb{nh}", bufs=2)
                nc.sync.dma_start(out=rb[:], in_=rt[D : D + 1, :, :])
                rbc = rpool.tile([D, nh, QB], F32, tag=f"rbc{nh}", bufs=2)
                nc.gpsimd.partition_broadcast(rbc[:], rb[:])
                for i, (h, outp) in enumerate(zip(hs, outps)):
                    nc.vector.tensor_mul(
                        out=attnO[:, h, qsl], in0=outp[0:D, :],
                        in1=rbc[:, i, :],
                    )

            # Software pipeline by emission order: attention(qb) interleaves
            # closures of outproj(qb-1) and proj(qb+1) between its groups so
            # the PE stream stays dense and no sequencer head-of-line blocks.
            for w in proj_work(0):
                w()
            pending = deque(proj_work(1) if NQB > 1 else [])
            for qb in range(NQB):
                nkt = 4 * qb + 4
                # h0 (base 0) and h1 (base 64) interleaved for PE row-group
                # concurrency; h2 follows solo with alternating base
                outp0 = opool.tile([D + 1, QB], F32, tag="outT", name=f"o0_{qb}")
                outp1 = opool.tile([D + 1, QB], F32, tag="outT", name=f"o1_{qb}")
                for g in range(nkt // 2):
                    attn_group(qb, 0, g, outp0, 0)
                    attn_group(qb, 1, g, outp1, 0)  # h1 tiles live at rows 64:
                    if pending:
                        pending.popleft()()
                normalize(qb, (0, 1), (outp0, outp1))
                outp2 = opool.tile([D + 1, QB], F32, tag="outT", name=f"o2_{qb}")
                for g in range(nkt // 2):
                    attn_group(qb, 2, g, outp2, (g % 2) * D)
                    if pending:
                        pending.popleft()()
                normalize(qb, (2,), (outp2,))
                while pending:
                    pending.popleft()()
                pending = deque(outproj_work(qb))
                if qb + 2 < NQB:
                    pending.extend(proj_work(qb + 2))
            while pending:
                pending.popleft()()


def _get_nc():
    if "nc" not in _CACHE:
        _CACHE["nc"] = _build_nc()
    return _CACHE["nc"]


def _shard_inputs(x, w_qkv, w_out):
    """Build per-core input maps."""
    x = np.asarray(x, dtype=np.float32)
    w_qkv = np.asarray(w_qkv, dtype=np.float32)
    w_out = np.asarray(w_out, dtype=np.float32)
    xTs = [np.ascontiguousarray(x[b].T) for b in range(B)]
    in_maps = []
    for c in range(NCORES):
        b = c // 4
        heads = [HPC * (c % 4) + i for i in range(HPC)]
        q = [w_qkv[:, h * D : (h + 1) * D] for h in heads]
        k = [w_qkv[:, C + h * D : C + (h + 1) * D] for h in heads]
        wqk = np.concatenate(
            [q[0], q[1], k[0], k[1], q[2], q[2], k[2], k[2]], axis=1
        )
        wv = np.zeros((C, 256), dtype=np.float32)
        for i, h in enumerate(heads):
            wv[:, i * (D + 1) : i * (D + 1) + D] = w_qkv[
                :, 2 * C + h * D : 2 * C + (h + 1) * D
            ]
        wo = np.stack(
            [w_out[h * D : (h + 1) * D, :] for h in heads], axis=1
        )  # [D, HPC, C]
        in_maps.append(
            {
                "xT": xTs[b],
                "wqk": np.ascontiguousarray(wqk),
                "wv": wv,
                "wo": np.ascontiguousarray(
                    wo.reshape(D, HPC * C).astype(bfloat16)
                ),
            }
        )
    return in_maps


def kernel(x, w_qkv, w_out, b_out):
    nc = _get_nc()
    in_maps = _shard_inputs(x, w_qkv, w_out)
    res = run_bass_kernel_spmd(nc, in_maps, core_ids=list(range(NCORES)))
    b_out = np.asarray(b_out, dtype=np.float32)
    outs = []
    for b in range(B):
        acc = res.results[4 * b]["out"].astype(np.float32).copy()
        for c in range(4 * b + 1, 4 * b + 4):
            acc += res.results[c]["out"]
        outs.append(acc + b_out[None, :])
    return np.stack(outs, axis=0)


# revision 12
# speedup vs baseline: 1.4181x; 1.1211x over previous
"""Causal self-attention (B=2, T=4096, C=768, H=12, D=64) on 8 trn2 cores.

Sharding: batch*heads across cores. Core c handles batch c//4 and heads
3*(c%4) .. 3*(c%4)+2. Each core computes the QKV projection for its head
slice, full causal attention for those heads, and a partial output
projection (its heads' rows of w_out). The host sums the 4 partials per
batch and adds b_out.

On-core layouts (q/k matmul operands float32r - fp32 consumed at full PE
rate with ~1e-4 rounding; v / softmax weights / out-proj in bf16):
  xT      [C, T]    input, pre-transposed on host
  tA      [128, T]  = [qT_h0 | qT_h1]   (rows 0:64 | 64:128)
  tB      [128, T]  = [kT_h0 | kT_h1]
  tQ2/tK2 [128, T]  = [qT_h2 | qT_h2] / [kT_h2 | kT_h2]  (duplicated
                      halves so h2's scores can use either partition
                      base without a cross-partition copy)
  vaug    [128, NKT, 256] bf16, v with a ones column per head at
                      col h*65+64 (so P@V also yields softmax denoms)
  scores  [128, 1024] psum, TWO k-tiles per bank-pair so one ACT exp
                      instruction covers 1024 columns (amortizes the
                      ~350-cycle ACT instruction overhead)
  attnO   [64, 3, T] bf16 normalized attention outputs per head

h0 lives at partition base 0 and h1 at base 64, and their score matmuls
are emitted adjacently, so the PE can run them concurrently in disjoint
row-groups (K=64 each).

Softmax denominators: PV psum row 64 = sum(exp) -> DVE reciprocal ->
gpsimd partition_broadcast -> DVE multiply. No DRAM round trips.

The causal mask is applied to the exp output of the 4 diagonal k-tiles
per q-block with gpsimd affine_select (fills 0 where q < k, including
the never-computed stale region of the staircase).
"""

import numpy as np
from ml_dtypes import bfloat16

import concourse.bass as bass
import concourse.mybir as mybir
import concourse.tile as tile
from concourse import bacc
from concourse.bass_utils import run_bass_kernel_spmd

B, T, C = 2, 4096, 768
NH, D = 12, 64
HPC = 3  # heads per core
NCORES = 8
P = 128
QB = 512           # q block == projection chunk
NQB = T // QB      # 8
NKT = T // P       # 32 k tiles
F32 = mybir.dt.float32
F32R = mybir.dt.float32r
BF16 = mybir.dt.bfloat16

_CACHE = {}


def _declare(nc):
    xT = nc.dram_tensor("xT", [C, T], BF16, kind="ExternalInput")
    wqk = nc.dram_tensor("wqk", [C, 4 * P], BF16, kind="ExternalInput")
    wv = nc.dram_tensor("wv", [C, 256], BF16, kind="ExternalInput")
    wo = nc.dram_tensor("wo", [D, HPC * C], BF16, kind="ExternalInput")
    out = nc.dram_tensor("out", [T, C], F32, kind="ExternalOutput")
    return dict(xT=xT.ap(), wqk=wqk.ap(), wv=wv.ap(), wo=wo.ap(), out=out.ap())


def _build_nc():
    nc = bacc.Bacc(
        "TRN2",
        target_bir_lowering=False,
        debug=False,
        enable_asserts=False,
        num_devices=NCORES,
    )
    aps = _declare(nc)
    with tile.TileContext(nc) as tc:
        _emit(tc, nc, **aps)
    nc.compile()
    return nc


def _emit(tc, nc, xT, wqk, wv, wo, out):
    import contextlib

    ctx = contextlib.ExitStack()
    with ctx:
        # ---- persistent sbuf ----
        persist = ctx.enter_context(tc.tile_pool(name="persist", bufs=1))
        tA = persist.tile([P, T], BF16, tag="tA")
        tB = persist.tile([P, T], BF16, tag="tB")
        tQ2 = persist.tile([P, T], BF16, tag="tQ2")
        tK2 = persist.tile([P, T], BF16, tag="tK2")
        vaug = persist.tile([P, NKT, 256], BF16, tag="vaug")
        attnO = persist.tile([D, HPC, T], BF16, tag="attnO")
        wqk_sb = persist.tile([P, 6, 4 * P], BF16, tag="wqk")
        wv_sb = persist.tile([P, 6, 256], BF16, tag="wv")
        wo_sb = persist.tile([D, HPC, C], BF16, tag="wo")
        ones_bf = persist.tile([P, 8], BF16, tag="ones")

        nc.sync.dma_start(out=wqk_sb[:], in_=wqk.rearrange("(co p) n -> p co n", p=P))
        nc.sync.dma_start(out=wv_sb[:], in_=wv.rearrange("(co p) n -> p co n", p=P))
        nc.sync.dma_start(out=wo_sb[:], in_=wo)
        nc.gpsimd.memset(ones_bf[:], 1.0)

        def qT(h, base=0):
            # h0 rows 0:64 of tA, h1 rows 64:128 of tA; h2 lives duplicated
            # in both halves of tQ2 so either partition base works
            return (tA[0:D], tA[D:P], tQ2[base : base + D])[h]

        def kT(h, base=0):
            return (tB[0:D], tB[D:P], tK2[base : base + D])[h]

        qk_dst = (tA, tB, tQ2, tK2)

        # psum budget (8 banks): p1 2 + scores 2*2 + outT 2
        with (
            tc.tile_pool(name="xchunks", bufs=2) as xpool,
            tc.tile_pool(name="p1psum", bufs=2, space="PSUM") as p1psum,
            tc.tile_pool(name="spsum", bufs=2, space="PSUM") as spool,
            tc.tile_pool(name="opsum", bufs=2, space="PSUM") as opool,
            tc.tile_pool(name="exps", bufs=3) as epool,
            tc.tile_pool(name="smalls", bufs=4) as rpool,
        ):
            from collections import deque

            def proj_work(qb):
                """Closures emitting projection chunk qb (tokens qb*512..)."""
                qsl = slice(qb * QB, (qb + 1) * QB)
                st = {}

                def dma():
                    xt = xpool.tile([P, 6, QB], BF16, tag="xt")
                    nc.sync.dma_start(
                        out=xt[:],
                        in_=xT[:, qsl].rearrange("(co p) t -> p co t", p=P),
                    )
                    st["xt"] = xt

                def chain(ci):
                    def f():
                        ps = p1psum.tile(
                            [P, QB], F32, tag="p1", name=f"p1_{qb}_{ci}"
                        )
                        for c6 in range(6):
                            nc.tensor.matmul(
                                ps[:],
                                wqk_sb[:, c6, ci * P : (ci + 1) * P],
                                st["xt"][:, c6, :],
                                start=(c6 == 0),
                                stop=(c6 == 5),
                            )
                        nc.vector.tensor_copy(out=qk_dst[ci][:, qsl], in_=ps[:])
                    return f

                def vhalf(half):
                    def f():
                        ktv = qb * (QB // P) + half
                        ps2 = p1psum.tile(
                            [P, QB], F32, tag="p1", name=f"p1v_{qb}_{half}"
                        )
                        for c6 in range(6):
                            nc.tensor.matmul(
                                ps2[:, 0:256],
                                st["xt"][:, c6, half * P : (half + 1) * P],
                                wv_sb[:, c6, :],
                                start=(c6 == 0),
                                stop=(c6 == 5),
                            )
                        nc.vector.tensor_copy(
                            out=vaug[:, ktv, :], in_=ps2[:, 0:256]
                        )
                        if half == QB // P - 1:
                            # restore the ones columns the v copies overwrote
                            for h in range(HPC):
                                nc.vector.tensor_copy(
                                    out=vaug[:, qb * (QB // P) :
                                             (qb + 1) * (QB // P),
                                             h * (D + 1) + D],
                                    in_=ones_bf[:, 0 : QB // P],
                                )
                    return f

                return (
                    [dma]
                    + [chain(ci) for ci in range(4)]
                    + [vhalf(h) for h in range(QB // P)]
                )

            def outproj_work(qb):
                """Closures emitting the output projection of q block qb."""
                def tt_work(tt):
                    def f():
                        tsl = slice(tt * P, (tt + 1) * P)
                        so = rpool.tile([P, C], F32, tag="p3out", bufs=2)
                        for noff, nsz in ((0, 512), (512, 256)):
                            po = p1psum.tile(
                                [P, QB], F32, tag="p1", name=f"po_{tt}_{noff}"
                            )
                            for h in range(HPC):
                                nc.tensor.matmul(
                                    po[:, :nsz],
                                    attnO[:, h, tsl],
                                    wo_sb[:, h, noff : noff + nsz],
                                    start=(h == 0),
                                    stop=(h == HPC - 1),
                                )
                            nc.vector.tensor_copy(
                                out=so[:, noff : noff + nsz], in_=po[:, :nsz]
                            )
                        nc.sync.dma_start(out=out[tsl, :], in_=so[:])
                    return f

                return [
                    tt_work(tt)
                    for tt in range(qb * (QB // P), (qb + 1) * (QB // P))
                ]

            def attn_group(qb, h, g, outp, base):
                """Two k-tiles (2g, 2g+1): scores -> one exp -> mask -> PV."""
                nkt = 4 * qb + 4
                sp = spool.tile([P, 2 * QB], F32, tag="sc",
                                name=f"sp_{qb}_{h}_{g}")
                for i in range(2):
                    kt = 2 * g + i
                    nc.tensor.matmul(
                        sp[:, i * QB : (i + 1) * QB],
                        kT(h, base)[:, kt * P : (kt + 1) * P],
                        qT(h, base)[:, qb * QB : (qb + 1) * QB],
                        start=True,
                        stop=True,
                    )
                ex = epool.tile([P, 2 * QB], BF16, tag="ex")
                nc.scalar.activation(
                    out=ex[:],
                    in_=sp[:],
                    func=mybir.ActivationFunctionType.Exp,
                    scale=float(D) ** -0.5,
                )
                for i in range(2):
                    kt = 2 * g + i
                    j = kt - 4 * qb
                    if j >= 0:  # diagonal tile: causal mask (fill 0 at q<k)
                        nc.gpsimd.affine_select(
                            out=ex[:, i * QB : (i + 1) * QB],
                            in_=ex[:, i * QB : (i + 1) * QB],
                            compare_op=mybir.AluOpType.is_ge,
                            fill=0.0,
                            base=-P * j,
                            pattern=[[1, QB]],
                            channel_multiplier=-1,
                        )
                for i in range(2):
                    kt = 2 * g + i
                    co = max(0, P * (kt - 4 * qb))
                    nc.tensor.matmul(
                        outp[:, co:],
                        vaug[:, kt, h * (D + 1) : (h + 1) * (D + 1)],
                        ex[:, i * QB + co : (i + 1) * QB],
                        start=(kt == 0),
                        stop=(kt == nkt - 1),
                    )

            def normalize(qb, hs, outps):
                """Softmax denominators for one or two heads at once: DVE
                reciprocal of psum row 64, one sbuf->sbuf DMA to move the
                rows to partition 0 (partition_broadcast's ucode reads
                physical partition 0 only), one gpsimd broadcast, then the
                normalizing multiplies."""
                qsl = slice(qb * QB, (qb + 1) * QB)
                nh = len(hs)
                rt = rpool.tile([D + 1, nh, QB], F32, tag=f"recip{nh}", bufs=2)
                for i, outp in enumerate(outps):
                    nc.vector.reciprocal(
                        out=rt[D : D + 1, i, :], in_=outp[D : D + 1, :]
                    )
                rb = rpool.tile([1, nh, QB], F32, tag=f"rb{nh}", bufs=2)
                nc.sync.dma_start(out=rb[:], in_=rt[D : D + 1, :, :])
                rbc = rpool.tile([D, nh, QB], F32, tag=f"rbc{nh}", bufs=2)
                nc.gpsimd.partition_broadcast(rbc[:], rb[:])
                for i, (h, outp) in enumerate(zip(hs, outps)):
                    nc.vector.tensor_mul(
                        out=attnO[:, h, qsl], in0=outp[0:D, :],
                        in1=rbc[:, i, :],
                    )

            # Software pipeline by emission order: attention(qb) interleaves
            # closures of outproj(qb-1) and proj(qb+1) between its groups so
            # the PE stream stays dense and no sequencer head-of-line blocks.
            for w in proj_work(0):
                w()
            pending = deque(proj_work(1) if NQB > 1 else [])
            for qb in range(NQB):
                nkt = 4 * qb + 4
                # h0 (base 0) and h1 (base 64) interleaved for PE row-group
                # concurrency; h2 follows solo with alternating base
                outp0 = opool.tile([D + 1, QB], F32, tag="outT", name=f"o0_{qb}")
                outp1 = opool.tile([D + 1, QB], F32, tag="outT", name=f"o1_{qb}")
                for g in range(nkt // 2):
                    attn_group(qb, 0, g, outp0, 0)
                    attn_group(qb, 1, g, outp1, 0)  # h1 tiles live at rows 64:
                    if pending:
                        pending.popleft()()
                normalize(qb, (0, 1), (outp0, outp1))
                outp2 = opool.tile([D + 1, QB], F32, tag="outT", name=f"o2_{qb}")
                for g in range(nkt // 2):
                    attn_group(qb, 2, g, outp2, (g % 2) * D)
                    if pending:
                        pending.popleft()()
                normalize(qb, (2,), (outp2,))
                while pending:
                    pending.popleft()()
                pending = deque(outproj_work(qb))
                if qb + 2 < NQB:
                    pending.extend(proj_work(qb + 2))
            while pending:
                pending.popleft()()


def _get_nc():
    if "nc" not in _CACHE:
        _CACHE["nc"] = _build_nc()
    return _CACHE["nc"]


def _shard_inputs(x, w_qkv, w_out):
    """Build per-core input maps."""
    x = np.asarray(x, dtype=np.float32)
    w_qkv = np.asarray(w_qkv, dtype=np.float32)
    w_out = np.asarray(w_out, dtype=np.float32)
    xTs = [np.ascontiguousarray(x[b].T) for b in range(B)]
    in_maps = []
    for c in range(NCORES):
        b = c // 4
        heads = [HPC * (c % 4) + i for i in range(HPC)]
        q = [w_qkv[:, h * D : (h + 1) * D] for h in heads]
        k = [w_qkv[:, C + h * D : C + (h + 1) * D] for h in heads]
        wqk = np.concatenate(
            [q[0], q[1], k[0], k[1], q[2], q[2], k[2], k[2]], axis=1
        )
        wv = np.zeros((C, 256), dtype=np.float32)
        for i, h in enumerate(heads):
            wv[:, i * (D + 1) : i * (D + 1) + D] = w_qkv[
                :, 2 * C + h * D : 2 * C + (h + 1) * D
            ]
        wo = np.stack(
            [w_out[h * D : (h + 1) * D, :] for h in heads], axis=1
        )  # [D, HPC, C]
        in_maps.append(
            {
                "xT": xTs[b].astype(bfloat16),
                "wqk": np.ascontiguousarray(wqk).astype(bfloat16),
                "wv": wv.astype(bfloat16),
                "wo": np.ascontiguousarray(
                    wo.reshape(D, HPC * C).astype(bfloat16)
                ),
            }
        )
    return in_maps


def kernel(x, w_qkv, w_out, b_out):
    nc = _get_nc()
    in_maps = _shard_inputs(x, w_qkv, w_out)
    res = run_bass_kernel_spmd(nc, in_maps, core_ids=list(range(NCORES)))
    b_out = np.asarray(b_out, dtype=np.float32)
    outs = []
    for b in range(B):
        acc = res.results[4 * b]["out"].astype(np.float32).copy()
        for c in range(4 * b + 1, 4 * b + 4):
            acc += res.results[c]["out"]
        outs.append(acc + b_out[None, :])
    return np.stack(outs, axis=0)


# revision 13
# speedup vs baseline: 1.4687x; 1.0357x over previous
"""Causal self-attention (B=2, T=4096, C=768, H=12, D=64) on 8 trn2 cores.

Sharding: batch*heads across cores. Core c handles batch c//4 and heads
3*(c%4) .. 3*(c%4)+2. Each core computes the QKV projection for its head
slice, full causal attention for those heads, and a partial output
projection (its heads' rows of w_out). The host sums the 4 partials per
batch and adds b_out.

On-core layouts (q/k matmul operands float32r - fp32 consumed at full PE
rate with ~1e-4 rounding; v / softmax weights / out-proj in bf16):
  xT      [C, T]    input, pre-transposed on host
  tA      [128, T]  = [qT_h0 | qT_h1]   (rows 0:64 | 64:128)
  tB      [128, T]  = [kT_h0 | kT_h1]
  tQ2/tK2 [128, T]  = [qT_h2 | qT_h2] / [kT_h2 | kT_h2]  (duplicated
                      halves so h2's scores can use either partition
                      base without a cross-partition copy)
  vaug    [128, NKT, 256] bf16, v with a ones column per head at
                      col h*65+64 (so P@V also yields softmax denoms)
  scores  [128, 1024] psum, TWO k-tiles per bank-pair so one ACT exp
                      instruction covers 1024 columns (amortizes the
                      ~350-cycle ACT instruction overhead)
  attnO   [64, 3, T] bf16 normalized attention outputs per head

h0 lives at partition base 0 and h1 at base 64, and their score matmuls
are emitted adjacently, so the PE can run them concurrently in disjoint
row-groups (K=64 each).

Softmax denominators: PV psum row 64 = sum(exp) -> DVE reciprocal ->
gpsimd partition_broadcast -> DVE multiply. No DRAM round trips.

The causal mask is applied to the exp output of the 4 diagonal k-tiles
per q-block with gpsimd affine_select (fills 0 where q < k, including
the never-computed stale region of the staircase).
"""

import numpy as np
from ml_dtypes import bfloat16

import concourse.bass as bass
import concourse.mybir as mybir
import concourse.tile as tile
from concourse import bacc
from concourse.bass_utils import run_bass_kernel_spmd

B, T, C = 2, 4096, 768
NH, D = 12, 64
HPC = 3  # heads per core
NCORES = 8
P = 128
QB = 512           # q block == projection chunk
NQB = T // QB      # 8
NKT = T // P       # 32 k tiles
F32 = mybir.dt.float32
F32R = mybir.dt.float32r
BF16 = mybir.dt.bfloat16

_CACHE = {}


def _declare(nc):
    xT = nc.dram_tensor("xT", [C, T], BF16, kind="ExternalInput")
    wqk = nc.dram_tensor("wqk", [C, 4 * P], BF16, kind="ExternalInput")
    wv = nc.dram_tensor("wv", [C, 256], BF16, kind="ExternalInput")
    wo = nc.dram_tensor("wo", [D, HPC * C], BF16, kind="ExternalInput")
    out = nc.dram_tensor("out", [T, C], F32, kind="ExternalOutput")
    return dict(xT=xT.ap(), wqk=wqk.ap(), wv=wv.ap(), wo=wo.ap(), out=out.ap())


def _build_nc():
    nc = bacc.Bacc(
        "TRN2",
        target_bir_lowering=False,
        debug=False,
        enable_asserts=False,
        num_devices=NCORES,
    )
    aps = _declare(nc)
    with tile.TileContext(nc) as tc:
        _emit(tc, nc, **aps)
    nc.compile()
    return nc


def _emit(tc, nc, xT, wqk, wv, wo, out):
    import contextlib

    ctx = contextlib.ExitStack()
    with ctx:
        # ---- persistent sbuf ----
        persist = ctx.enter_context(tc.tile_pool(name="persist", bufs=1))
        tA = persist.tile([P, T], BF16, tag="tA")
        tB = persist.tile([P, T], BF16, tag="tB")
        tQ2 = persist.tile([P, T], BF16, tag="tQ2")
        tK2 = persist.tile([P, T], BF16, tag="tK2")
        vaug = persist.tile([P, NKT, 256], BF16, tag="vaug")
        attnO = persist.tile([D, HPC, T], BF16, tag="attnO")
        wqk_sb = persist.tile([P, 6, 4 * P], BF16, tag="wqk")
        wv_sb = persist.tile([P, 6, 256], BF16, tag="wv")
        wo_sb = persist.tile([D, HPC, C], BF16, tag="wo")
        ones_bf = persist.tile([P, 8], BF16, tag="ones")

        nc.sync.dma_start(out=wqk_sb[:], in_=wqk.rearrange("(co p) n -> p co n", p=P))
        nc.sync.dma_start(out=wv_sb[:], in_=wv.rearrange("(co p) n -> p co n", p=P))
        nc.sync.dma_start(out=wo_sb[:], in_=wo)
        nc.gpsimd.memset(ones_bf[:], 1.0)

        def qT(h, base=0):
            # h0 rows 0:64 of tA, h1 rows 64:128 of tA; h2 lives duplicated
            # in both halves of tQ2 so either partition base works
            return (tA[0:D], tA[D:P], tQ2[base : base + D])[h]

        def kT(h, base=0):
            return (tB[0:D], tB[D:P], tK2[base : base + D])[h]

        qk_dst = (tA, tB, tQ2, tK2)

        # psum budget (8 banks): p1 2 + scores 2*2 + outT 2
        with (
            tc.tile_pool(name="xchunks", bufs=2) as xpool,
            tc.tile_pool(name="p1psum", bufs=2, space="PSUM") as p1psum,
            tc.tile_pool(name="spsum", bufs=2, space="PSUM") as spool,
            tc.tile_pool(name="opsum", bufs=2, space="PSUM") as opool,
            tc.tile_pool(name="exps", bufs=3) as epool,
            tc.tile_pool(name="smalls", bufs=4) as rpool,
        ):
            from collections import deque

            def proj_work(qb):
                """Closures emitting projection chunk qb (tokens qb*512..)."""
                qsl = slice(qb * QB, (qb + 1) * QB)
                st = {}

                def dma():
                    xt = xpool.tile([P, 6, QB], BF16, tag="xt")
                    nc.sync.dma_start(
                        out=xt[:],
                        in_=xT[:, qsl].rearrange("(co p) t -> p co t", p=P),
                    )
                    st["xt"] = xt

                def chain(ci):
                    def f():
                        ps = p1psum.tile(
                            [P, QB], F32, tag="p1", name=f"p1_{qb}_{ci}"
                        )
                        for c6 in range(6):
                            nc.tensor.matmul(
                                ps[:],
                                wqk_sb[:, c6, ci * P : (ci + 1) * P],
                                st["xt"][:, c6, :],
                                start=(c6 == 0),
                                stop=(c6 == 5),
                            )
                        nc.vector.tensor_copy(out=qk_dst[ci][:, qsl], in_=ps[:])
                    return f

                def vhalf(half):
                    def f():
                        ktv = qb * (QB // P) + half
                        ps2 = p1psum.tile(
                            [P, QB], F32, tag="p1", name=f"p1v_{qb}_{half}"
                        )
                        for c6 in range(6):
                            nc.tensor.matmul(
                                ps2[:, 0:256],
                                st["xt"][:, c6, half * P : (half + 1) * P],
                                wv_sb[:, c6, :],
                                start=(c6 == 0),
                                stop=(c6 == 5),
                            )
                        nc.vector.tensor_copy(
                            out=vaug[:, ktv, :], in_=ps2[:, 0:256]
                        )
                        if half == QB // P - 1:
                            # restore the ones columns the v copies overwrote
                            for h in range(HPC):
                                nc.vector.tensor_copy(
                                    out=vaug[:, qb * (QB // P) :
                                             (qb + 1) * (QB // P),
                                             h * (D + 1) + D],
                                    in_=ones_bf[:, 0 : QB // P],
                                )
                    return f

                return (
                    [dma]
                    + [chain(ci) for ci in range(4)]
                    + [vhalf(h) for h in range(QB // P)]
                )

            def outproj_work(qb):
                """Closures emitting the output projection of q block qb."""
                def tt_work(tt):
                    def f():
                        tsl = slice(tt * P, (tt + 1) * P)
                        so = rpool.tile([P, C], F32, tag="p3out", bufs=2)
                        for noff, nsz in ((0, 512), (512, 256)):
                            po = p1psum.tile(
                                [P, QB], F32, tag="p1", name=f"po_{tt}_{noff}"
                            )
                            for h in range(HPC):
                                nc.tensor.matmul(
                                    po[:, :nsz],
                                    attnO[:, h, tsl],
                                    wo_sb[:, h, noff : noff + nsz],
                                    start=(h == 0),
                                    stop=(h == HPC - 1),
                                )
                            nc.vector.tensor_copy(
                                out=so[:, noff : noff + nsz], in_=po[:, :nsz]
                            )
                        nc.sync.dma_start(out=out[tsl, :], in_=so[:])
                    return f

                return [
                    tt_work(tt)
                    for tt in range(qb * (QB // P), (qb + 1) * (QB // P))
                ]

            def attn_group(qb, h, g, outp, base):
                """Two k-tiles (2g, 2g+1): scores -> one exp -> mask -> PV."""
                nkt = 4 * qb + 4
                sp = spool.tile([P, 2 * QB], F32, tag="sc",
                                name=f"sp_{qb}_{h}_{g}")
                for i in range(2):
                    kt = 2 * g + i
                    nc.tensor.matmul(
                        sp[:, i * QB : (i + 1) * QB],
                        kT(h, base)[:, kt * P : (kt + 1) * P],
                        qT(h, base)[:, qb * QB : (qb + 1) * QB],
                        start=True,
                        stop=True,
                    )
                ex = epool.tile([P, 2 * QB], BF16, tag="ex")
                nc.scalar.activation(
                    out=ex[:],
                    in_=sp[:],
                    func=mybir.ActivationFunctionType.Exp,
                    scale=float(D) ** -0.5,
                )
                for i in range(2):
                    kt = 2 * g + i
                    j = kt - 4 * qb
                    if j >= 0:  # diagonal tile: causal mask (fill 0 at q<k)
                        nc.gpsimd.affine_select(
                            out=ex[:, i * QB : (i + 1) * QB],
                            in_=ex[:, i * QB : (i + 1) * QB],
                            compare_op=mybir.AluOpType.is_ge,
                            fill=0.0,
                            base=-P * j,
                            pattern=[[1, QB]],
                            channel_multiplier=-1,
                        )
                for i in range(2):
                    kt = 2 * g + i
                    co = max(0, P * (kt - 4 * qb))
                    nc.tensor.matmul(
                        outp[:, co:],
                        vaug[:, kt, h * (D + 1) : (h + 1) * (D + 1)],
                        ex[:, i * QB + co : (i + 1) * QB],
                        start=(kt == 0),
                        stop=(kt == nkt - 1),
                    )

            def normalize(qb, hs, outps):
                """Softmax denominators for one or two heads at once: DVE
                reciprocal of psum row 64, one sbuf->sbuf DMA to move the
                rows to partition 0 (partition_broadcast's ucode reads
                physical partition 0 only), one gpsimd broadcast, then the
                normalizing multiplies."""
                qsl = slice(qb * QB, (qb + 1) * QB)
                nh = len(hs)
                # copy PV psum to sbuf promptly so the opsum bank frees for
                # the next head's accumulation; normalize runs off sbuf
                ot = rpool.tile([D + 1, nh, QB], F32, tag=f"ot{nh}", bufs=2)
                for i, outp in enumerate(outps):
                    nc.vector.tensor_copy(out=ot[:, i, :], in_=outp[:])
                rt = rpool.tile([D + 1, nh, QB], F32, tag=f"recip{nh}", bufs=1)
                nc.vector.reciprocal(
                    out=rt[D : D + 1, :, :], in_=ot[D : D + 1, :, :]
                )
                rb = rpool.tile([1, nh, QB], F32, tag=f"rb{nh}", bufs=1)
                nc.sync.dma_start(out=rb[:], in_=rt[D : D + 1, :, :])
                rbc = rpool.tile([D, nh, QB], F32, tag=f"rbc{nh}", bufs=1)
                nc.gpsimd.partition_broadcast(rbc[:], rb[:])
                for i, h in enumerate(hs):
                    nc.vector.tensor_mul(
                        out=attnO[:, h, qsl], in0=ot[0:D, i, :],
                        in1=rbc[:, i, :],
                    )

            # Software pipeline by emission order: attention(qb) interleaves
            # closures of outproj(qb-1) and proj(qb+1) between its groups so
            # the PE stream stays dense and no sequencer head-of-line blocks.
            for w in proj_work(0):
                w()
            pending = deque(proj_work(1) if NQB > 1 else [])
            for qb in range(NQB):
                nkt = 4 * qb + 4
                # h0 (base 0) and h1 (base 64) interleaved for PE row-group
                # concurrency; h2 follows solo with alternating base
                outp0 = opool.tile([D + 1, QB], F32, tag="outT", name=f"o0_{qb}")
                outp1 = opool.tile([D + 1, QB], F32, tag="outT", name=f"o1_{qb}")
                for g in range(nkt // 2):
                    attn_group(qb, 0, g, outp0, 0)
                    attn_group(qb, 1, g, outp1, 0)  # h1 tiles live at rows 64:
                    if pending:
                        pending.popleft()()
                normalize(qb, (0, 1), (outp0, outp1))
                outp2 = opool.tile([D + 1, QB], F32, tag="outT", name=f"o2_{qb}")
                for g in range(nkt // 2):
                    attn_group(qb, 2, g, outp2, (g % 2) * D)
                    if pending:
                        pending.popleft()()
                normalize(qb, (2,), (outp2,))
                while pending:
                    pending.popleft()()
                pending = deque(outproj_work(qb))
                if qb + 2 < NQB:
                    pending.extend(proj_work(qb + 2))
            while pending:
                pending.popleft()()


def _get_nc():
    if "nc" not in _CACHE:
        _CACHE["nc"] = _build_nc()
    return _CACHE["nc"]


def _shard_inputs(x, w_qkv, w_out):
    """Build per-core input maps."""
    x = np.asarray(x, dtype=np.float32)
    w_qkv = np.asarray(w_qkv, dtype=np.float32)
    w_out = np.asarray(w_out, dtype=np.float32)
    xTs = [np.ascontiguousarray(x[b].T) for b in range(B)]
    in_maps = []
    for c in range(NCORES):
        b = c // 4
        heads = [HPC * (c % 4) + i for i in range(HPC)]
        q = [w_qkv[:, h * D : (h + 1) * D] for h in heads]
        k = [w_qkv[:, C + h * D : C + (h + 1) * D] for h in heads]
        wqk = np.concatenate(
            [q[0], q[1], k[0], k[1], q[2], q[2], k[2], k[2]], axis=1
        )
        wv = np.zeros((C, 256), dtype=np.float32)
        for i, h in enumerate(heads):
            wv[:, i * (D + 1) : i * (D + 1) + D] = w_qkv[
                :, 2 * C + h * D : 2 * C + (h + 1) * D
            ]
        wo = np.stack(
            [w_out[h * D : (h + 1) * D, :] for h in heads], axis=1
        )  # [D, HPC, C]
        in_maps.append(
            {
                "xT": xTs[b].astype(bfloat16),
                "wqk": np.ascontiguousarray(wqk).astype(bfloat16),
                "wv": wv.astype(bfloat16),
                "wo": np.ascontiguousarray(
                    wo.reshape(D, HPC * C).astype(bfloat16)
                ),
            }
        )
    return in_maps


def kernel(x, w_qkv, w_out, b_out):
    nc = _get_nc()
    in_maps = _shard_inputs(x, w_qkv, w_out)
    res = run_bass_kernel_spmd(nc, in_maps, core_ids=list(range(NCORES)))
    b_out = np.asarray(b_out, dtype=np.float32)
    outs = []
    for b in range(B):
        acc = res.results[4 * b]["out"].astype(np.float32).copy()
        for c in range(4 * b + 1, 4 * b + 4):
            acc += res.results[c]["out"]
        outs.append(acc + b_out[None, :])
    return np.stack(outs, axis=0)


# revision 19
# speedup vs baseline: 1.5006x; 1.0217x over previous
"""Causal self-attention (B=2, T=4096, C=768, H=12, D=64) on 8 trn2 cores.

Sharding: batch*heads across cores. Core c handles batch c//4 and heads
3*(c%4) .. 3*(c%4)+2. Each core computes the QKV projection for its head
slice, full causal attention for those heads, and a partial output
projection (its heads' rows of w_out). The host sums the 4 partials per
batch and adds b_out.

On-core layouts (q/k matmul operands float32r - fp32 consumed at full PE
rate with ~1e-4 rounding; v / softmax weights / out-proj in bf16):
  xT      [C, T]    input, pre-transposed on host
  tA      [128, T]  = [qT_h0 | qT_h1]   (rows 0:64 | 64:128)
  tB      [128, T]  = [kT_h0 | kT_h1]
  tQ2/tK2 [128, T]  = [qT_h2 | qT_h2] / [kT_h2 | kT_h2]  (duplicated
                      halves so h2's scores can use either partition
                      base without a cross-partition copy)
  vaug    [128, NKT, 256] bf16, v with a ones column per head at
                      col h*65+64 (so P@V also yields softmax denoms)
  scores  [128, 1024] psum, TWO k-tiles per bank-pair so one ACT exp
                      instruction covers 1024 columns (amortizes the
                      ~350-cycle ACT instruction overhead)
  attnO   [64, 3, T] bf16 normalized attention outputs per head

h0 lives at partition base 0 and h1 at base 64, and their score matmuls
are emitted adjacently, so the PE can run them concurrently in disjoint
row-groups (K=64 each).

Softmax denominators: PV psum row 64 = sum(exp) -> DVE reciprocal ->
gpsimd partition_broadcast -> DVE multiply. No DRAM round trips.

The causal mask is applied to the exp output of the 4 diagonal k-tiles
per q-block with gpsimd affine_select (fills 0 where q < k, including
the never-computed stale region of the staircase).
"""

import numpy as np
from ml_dtypes import bfloat16

import concourse.bass as bass
import concourse.mybir as mybir
import concourse.tile as tile
from concourse import bacc
from concourse.bass_utils import run_bass_kernel_spmd

B, T, C = 2, 4096, 768
NH, D = 12, 64
HPC = 3  # heads per core
NCORES = 8
P = 128
QB = 512           # q block == projection chunk
NQB = T // QB      # 8
NKT = T // P       # 32 k tiles
F32 = mybir.dt.float32
F32R = mybir.dt.float32r
BF16 = mybir.dt.bfloat16

_CACHE = {}


def _declare(nc):
    xT = nc.dram_tensor("xT", [C, T], BF16, kind="ExternalInput")
    wqk = nc.dram_tensor("wqk", [C, 4 * P], BF16, kind="ExternalInput")
    wv = nc.dram_tensor("wv", [C, 256], BF16, kind="ExternalInput")
    wo = nc.dram_tensor("wo", [HPC * D, C], BF16, kind="ExternalInput")
    out = nc.dram_tensor("out", [T, C], F32, kind="ExternalOutput")
    return dict(xT=xT.ap(), wqk=wqk.ap(), wv=wv.ap(), wo=wo.ap(), out=out.ap())


def _build_nc():
    nc = bacc.Bacc(
        "TRN2",
        target_bir_lowering=False,
        debug=False,
        enable_asserts=False,
        num_devices=NCORES,
    )
    aps = _declare(nc)
    with tile.TileContext(nc) as tc:
        _emit(tc, nc, **aps)
    nc.compile()
    return nc


def _emit(tc, nc, xT, wqk, wv, wo, out):
    import contextlib

    ctx = contextlib.ExitStack()
    with ctx:
        # ---- persistent sbuf ----
        persist = ctx.enter_context(tc.tile_pool(name="persist", bufs=1))
        tA = persist.tile([P, T], BF16, tag="tA")
        tB = persist.tile([P, T], BF16, tag="tB")
        tQ2 = persist.tile([P, T], BF16, tag="tQ2")
        tK2 = persist.tile([P, T], BF16, tag="tK2")
        vaug = persist.tile([P, NKT, 256], BF16, tag="vaug")
        attnOAB = persist.tile([P, T], BF16, tag="attnOAB")
        attnO2 = persist.tile([D, T], BF16, tag="attnO2")
        wqk_sb = persist.tile([P, 6, 4 * P], BF16, tag="wqk")
        wv_sb = persist.tile([P, 6, 256], BF16, tag="wv")
        woAB_sb = persist.tile([P, C], BF16, tag="woAB")
        wo2_sb = persist.tile([D, C], BF16, tag="wo2")
        ones_bf = persist.tile([P, 8], BF16, tag="ones")

        nc.sync.dma_start(out=wqk_sb[:], in_=wqk.rearrange("(co p) n -> p co n", p=P))
        nc.sync.dma_start(out=wv_sb[:], in_=wv.rearrange("(co p) n -> p co n", p=P))
        nc.sync.dma_start(out=woAB_sb[:], in_=wo[0:P, :])
        nc.sync.dma_start(out=wo2_sb[:], in_=wo[P : P + D, :])
        nc.gpsimd.memset(ones_bf[:], 1.0)

        def qT(h, base=0):
            # h0 rows 0:64 of tA, h1 rows 64:128 of tA; h2 lives duplicated
            # in both halves of tQ2 so either partition base works
            return (tA[0:D], tA[D:P], tQ2[base : base + D])[h]

        def kT(h, base=0):
            return (tB[0:D], tB[D:P], tK2[base : base + D])[h]

        qk_dst = (tA, tB, tQ2, tK2)

        # psum budget (8 banks): p1 2 + scores 2*2 + outT 2
        with (
            tc.tile_pool(name="xchunks", bufs=2) as xpool,
            tc.tile_pool(name="p1psum", bufs=2, space="PSUM") as p1psum,
            tc.tile_pool(name="spsum", bufs=2, space="PSUM") as spool,
            tc.tile_pool(name="opsum", bufs=2, space="PSUM") as opool,
            tc.tile_pool(name="exps", bufs=3) as epool,
            tc.tile_pool(name="smalls", bufs=4) as rpool,
        ):
            from collections import deque

            def proj_work(qb):
                """Closures emitting projection chunk qb (tokens qb*512..)."""
                qsl = slice(qb * QB, (qb + 1) * QB)
                st = {}

                def dma():
                    xt = xpool.tile([P, 6, QB], BF16, tag="xt")
                    nc.sync.dma_start(
                        out=xt[:],
                        in_=xT[:, qsl].rearrange("(co p) t -> p co t", p=P),
                    )
                    st["xt"] = xt

                def chain(ci):
                    def f():
                        ps = p1psum.tile(
                            [P, QB], F32, tag="p1", name=f"p1_{qb}_{ci}"
                        )
                        for c6 in range(6):
                            nc.tensor.matmul(
                                ps[:],
                                wqk_sb[:, c6, ci * P : (ci + 1) * P],
                                st["xt"][:, c6, :],
                                start=(c6 == 0),
                                stop=(c6 == 5),
                            )
                        nc.vector.tensor_copy(out=qk_dst[ci][:, qsl], in_=ps[:])
                    return f

                def vhalf(half):
                    def f():
                        ktv = qb * (QB // P) + half
                        ps2 = p1psum.tile(
                            [P, QB], F32, tag="p1", name=f"p1v_{qb}_{half}"
                        )
                        for c6 in range(6):
                            nc.tensor.matmul(
                                ps2[:, 0:256],
                                st["xt"][:, c6, half * P : (half + 1) * P],
                                wv_sb[:, c6, :],
                                start=(c6 == 0),
                                stop=(c6 == 5),
                            )
                        nc.vector.tensor_copy(
                            out=vaug[:, ktv, :], in_=ps2[:, 0:256]
                        )
                        if half == QB // P - 1:
                            # restore the ones columns the v copies overwrote
                            for h in range(HPC):
                                nc.vector.tensor_copy(
                                    out=vaug[:, qb * (QB // P) :
                                             (qb + 1) * (QB // P),
                                             h * (D + 1) + D],
                                    in_=ones_bf[:, 0 : QB // P],
                                )
                    return f

                return (
                    [dma]
                    + [chain(ci) for ci in range(4)]
                    + [vhalf(h) for h in range(QB // P)]
                )

            def outproj_work(qb):
                """Closures emitting the output projection of q block qb."""
                def tt_work(tt):
                    def f():
                        tsl = slice(tt * P, (tt + 1) * P)
                        so = rpool.tile([P, C], F32, tag="p3out", bufs=2)
                        for noff, nsz in ((0, 512), (512, 256)):
                            po = p1psum.tile(
                                [P, QB], F32, tag="p1", name=f"po_{tt}_{noff}"
                            )
                            nc.tensor.matmul(
                                po[:, :nsz],
                                attnOAB[:, tsl],
                                woAB_sb[:, noff : noff + nsz],
                                start=True,
                                stop=False,
                            )
                            nc.tensor.matmul(
                                po[:, :nsz],
                                attnO2[:, tsl],
                                wo2_sb[:, noff : noff + nsz],
                                start=False,
                                stop=True,
                            )
                            nc.vector.tensor_copy(
                                out=so[:, noff : noff + nsz], in_=po[:, :nsz]
                            )
                        nc.sync.dma_start(out=out[tsl, :], in_=so[:])
                    return f

                return [
                    tt_work(tt)
                    for tt in range(qb * (QB // P), (qb + 1) * (QB // P))
                ]

            def attn_group(qb, g, hb, outps):
                """Two k-tiles (2g, 2g+1) for one or two heads: interleaved
                scores (consecutive matmuls always hit alternating PE
                row-groups, which run concurrently; same-row-group
                back-to-back K=64 matmuls serialize their weight loads and
                cost ~2.3x) -> one exp per head -> mask -> PV."""
                nkt = 4 * qb + 4
                hb = list(hb)
                sps, exs = [], []
                for h, _ in hb:
                    sps.append(spool.tile([P, 2 * QB], F32, tag="sc",
                                          name=f"sp_{qb}_{h}_{g}"))
                for i in range(2):
                    kt = 2 * g + i
                    for (h, bases), sp in zip(hb, sps):
                        base = bases[i]
                        nc.tensor.matmul(
                            sp[:, i * QB : (i + 1) * QB],
                            kT(h, base)[:, kt * P : (kt + 1) * P],
                            qT(h, base)[:, qb * QB : (qb + 1) * QB],
                            start=True,
                            stop=True,
                        )
                for sp in sps:
                    ex = epool.tile([P, 2 * QB], BF16, tag="ex")
                    nc.scalar.activation(
                        out=ex[:],
                        in_=sp[:],
                        func=mybir.ActivationFunctionType.Exp,
                        scale=float(D) ** -0.5,
                    )
                    exs.append(ex)
                for i in range(2):
                    kt = 2 * g + i
                    j = kt - 4 * qb
                    if j >= 0:  # diagonal tile: causal mask (fill 0 at q<k)
                        for ex in exs:
                            nc.gpsimd.affine_select(
                                out=ex[:, i * QB : (i + 1) * QB],
                                in_=ex[:, i * QB : (i + 1) * QB],
                                compare_op=mybir.AluOpType.is_ge,
                                fill=0.0,
                                base=-P * j,
                                pattern=[[1, QB]],
                                channel_multiplier=-1,
                            )
                for ((h, _), ex, outp) in zip(hb, exs, outps):
                    for i in range(2):
                        kt = 2 * g + i
                        co = max(0, P * (kt - 4 * qb))
                        nc.tensor.matmul(
                            outp[:, co:],
                            vaug[:, kt, h * (D + 1) : (h + 1) * (D + 1)],
                            ex[:, i * QB + co : (i + 1) * QB],
                            start=(kt == 0),
                            stop=(kt == nkt - 1),
                        )

            def normalize(qb, hs, outps):
                """Softmax denominators for one or two heads at once: DVE
                reciprocal of psum row 64, one sbuf->sbuf DMA to move the
                rows to partition 0 (partition_broadcast's ucode reads
                physical partition 0 only), one gpsimd broadcast, then the
                normalizing multiplies."""
                qsl = slice(qb * QB, (qb + 1) * QB)
                nh = len(hs)
                # copy PV psum to sbuf promptly so the opsum bank frees for
                # the next head's accumulation; normalize runs off sbuf
                ot = rpool.tile([D + 1, nh, QB], F32, tag=f"ot{nh}", bufs=2)
                for i, outp in enumerate(outps):
                    nc.vector.tensor_copy(out=ot[:, i, :], in_=outp[:])
                rt = rpool.tile([D + 1, nh, QB], F32, tag=f"recip{nh}", bufs=1)
                nc.vector.reciprocal(
                    out=rt[D : D + 1, :, :], in_=ot[D : D + 1, :, :]
                )
                rb = rpool.tile([1, nh, QB], F32, tag=f"rb{nh}", bufs=1)
                nc.sync.dma_start(out=rb[:], in_=rt[D : D + 1, :, :])
                rbc = rpool.tile([D, nh, QB], F32, tag=f"rbc{nh}", bufs=1)
                nc.gpsimd.partition_broadcast(rbc[:], rb[:])
                for i, h in enumerate(hs):
                    if h == 0:
                        dst = attnOAB[0:D, qsl]
                    elif h == 2:
                        dst = attnO2[:, qsl]
                    else:
                        # h1 belongs at partitions 64:128 of attnOAB, which
                        # engines cannot reach from lanes 0:64 - stage and
                        # DMA-bounce (sbuf->sbuf)
                        sg = rpool.tile([D, QB], BF16, tag="sg", bufs=2,
                                        name=f"sg_{qb}")
                        dst = sg[:]
                    nc.vector.tensor_mul(
                        out=dst, in0=ot[0:D, i, :], in1=rbc[:, i, :]
                    )
                    if h == 1:
                        nc.sync.dma_start(out=attnOAB[D:P, qsl], in_=dst)

            # Software pipeline by emission order: attention(qb) interleaves
            # closures of outproj(qb-1) and proj(qb+1) between its groups so
            # the PE stream stays dense and no sequencer head-of-line blocks.
            for w in proj_work(0):
                w()
            pending = deque(proj_work(1) if NQB > 1 else [])
            for qb in range(NQB):
                nkt = 4 * qb + 4
                # h0 (base 0) and h1 (base 64) interleaved for PE row-group
                # concurrency; h2 follows solo with alternating base
                outp0 = opool.tile([D + 1, QB], F32, tag="outT", name=f"o0_{qb}")
                outp1 = opool.tile([D + 1, QB], F32, tag="outT", name=f"o1_{qb}")
                for g in range(nkt // 2):
                    attn_group(qb, g, [(0, (0, 0)), (1, (0, 0))],
                               [outp0, outp1])
                    if pending:
                        pending.popleft()()
                normalize(qb, (0, 1), (outp0, outp1))
                outp2 = opool.tile([D + 1, QB], F32, tag="outT", name=f"o2_{qb}")
                for g in range(nkt // 2):
                    attn_group(qb, g, [(2, (0, D))], [outp2])
                    if pending:
                        pending.popleft()()
                normalize(qb, (2,), (outp2,))
                while pending:
                    pending.popleft()()
                pending = deque(outproj_work(qb))
                if qb + 2 < NQB:
                    pending.extend(proj_work(qb + 2))
            while pending:
                pending.popleft()()


def _get_nc():
    if "nc" not in _CACHE:
        _CACHE["nc"] = _build_nc()
    return _CACHE["nc"]


def _shard_inputs(x, w_qkv, w_out):
    """Build per-core input maps."""
    x = np.asarray(x, dtype=np.float32)
    w_qkv = np.asarray(w_qkv, dtype=np.float32)
    w_out = np.asarray(w_out, dtype=np.float32)
    xTs = [np.ascontiguousarray(x[b].T) for b in range(B)]
    in_maps = []
    for c in range(NCORES):
        b = c // 4
        heads = [HPC * (c % 4) + i for i in range(HPC)]
        q = [w_qkv[:, h * D : (h + 1) * D] for h in heads]
        k = [w_qkv[:, C + h * D : C + (h + 1) * D] for h in heads]
        wqk = np.concatenate(
            [q[0], q[1], k[0], k[1], q[2], q[2], k[2], k[2]], axis=1
        )
        wv = np.zeros((C, 256), dtype=np.float32)
        for i, h in enumerate(heads):
            wv[:, i * (D + 1) : i * (D + 1) + D] = w_qkv[
                :, 2 * C + h * D : 2 * C + (h + 1) * D
            ]
        wo = np.concatenate(
            [w_out[h * D : (h + 1) * D, :] for h in heads], axis=0
        )  # [HPC*D, C]
        in_maps.append(
            {
                "xT": xTs[b].astype(bfloat16),
                "wqk": np.ascontiguousarray(wqk).astype(bfloat16),
                "wv": wv.astype(bfloat16),
                "wo": np.ascontiguousarray(wo).astype(bfloat16),
            }
        )
    return in_maps


def kernel(x, w_qkv, w_out, b_out):
    nc = _get_nc()
    in_maps = _shard_inputs(x, w_qkv, w_out)
    res = run_bass_kernel_spmd(nc, in_maps, core_ids=list(range(NCORES)))
    b_out = np.asarray(b_out, dtype=np.float32)
    outs = []
    for b in range(B):
        acc = res.results[4 * b]["out"].astype(np.float32).copy()
        for c in range(4 * b + 1, 4 * b + 4):
            acc += res.results[c]["out"]
        outs.append(acc + b_out[None, :])
    return np.stack(outs, axis=0)
